# revision 33
# baseline (speedup 1.0000x reference)
"""Trainium2 Bass kernel for nn_AttentionHead (B=8, N=2048, D=512, d=64).

Reference semantics (faithful to the torch original):
    K = key_input   @ W_key        # note: W_key used for Q, K AND V
    Q = query_input @ W_key
    V = value_input @ W_key
    S = Q @ K^T / sqrt(512)        # scaled by INPUT dim, not head dim
    S = mask(padding), causal-mask if masked_attention
    out = softmax(S) @ V

Sharding: pure data parallelism over batch — core b computes batch element b.
No collectives. Host-side prep is layout only (transpose + dtype cast +
output unpermute); every FLOP of the math runs on-device.

Device algorithm (per core), v9:
  - xq/xk stream in fp8e4 (W_qk prescaled x16 host-side, descale folded into
    the exp scale); projections use DoubleRow perf mode (256-deep
    contraction, half the matmuls); xv stays bf16 to protect output precision
  - DMA completion is DESCRIPTOR-bound (~130ns per partition line), so all
    inputs are host-packed into 4KB-per-partition lines: q/k slices ride in
    PAIRS, v slices whole. Priority is enforced with real data deps only
    (corner-gate DMAs + x-pool buf reuse) — the rings round-robin across
    all in-flight transfers and the tile scheduler reorders issue order
  - q-block-outer attention: per 512-wide q-block, k-chunks processed in
    row-packed pairs (two 64-deep S matmuls concurrently in disjoint PE row
    groups) writing one [128, 1024] PSUM tile; ONE wide exp per pair on ACT
    (amortizes the ~293ns ACTIVATE overhead); p_sb pool sized one-per-group
    so no recycle deps serialize the ACT queue
  - diagonal k-chunks compute S full-width (free under row-pairing) so the
    wide exp read is contiguous; only 128-wide diagonal blocks get the
    upper-triangular mask multiply (DVE early, gpsimd for late q-blocks);
    PV uses exact causal widths
  - O.T [65, q] accumulated in PSUM over k-chunks (ones column appended to
    V-natural gives softmax denominators as row 64); per-q-block epilogue:
    PE-transpose, reciprocal, scale, one batched DMA out
  - PSUM: 2x [128,1024] S (4 banks) + 2x [65,512] O (2) + 2x [128,512]
    proj/transpose (2) = 8 banks
"""

import math

import numpy as np
import ml_dtypes

import concourse.bass as bass
import concourse.tile as tile
from concourse import bacc, mybir
from concourse import masks
from concourse.bass_utils import run_bass_kernel_spmd

P = 128            # partitions / k-chunk size
N = 2048           # sequence length
D = 512            # embedding dim
DH = 64            # head dim
EC = D // P        # 4 e-chunks for the (bf16) V projection contraction
EC2 = D // (2 * P)  # 2 double-row chunks for the fp8 Q/K projections
KC = N // P        # 16 k-chunks
QW = 512           # q block width
NQB = N // QW      # 4 q blocks / n slices
WS = 16.0          # host-side W_qk prescale (fp8 range use)
SCALE = 1.0 / math.sqrt(float(D))
EXP_SCALE = SCALE / (WS * WS)

BF16 = mybir.dt.bfloat16
FP8 = mybir.dt.float8e4
F32 = mybir.dt.float32
DR = mybir.MatmulPerfMode.DoubleRow

_BUILD_CACHE = {}

OPTS = {
    "pe_warm": 30,     # dummy matmuls at t=0 to lift the HAM clock gate
    "ppool": 20,       # p_sb wide-tile buffers: one per group, so no recycle
                       # deps ever land as EVENT_SEMAPHORE waits on ACT
    "use_dr": True,    # DoubleRow fp8 projections
}


def _ensure_ntff_hook():
    """Install the antenv.axon_hooks shim so trace=True works under axon."""
    try:
        import antenv.axon_hooks  # noqa: F401
        return
    except ImportError:
        pass
    import sys
    import types

    try:
        from trn_agent_boot.trn_boot import _ntff_profile_via_ctypes
        hook = _ntff_profile_via_ctypes("/opt/axon/libaxon_pjrt.so")
    except Exception:
        hook = None
    mod = types.ModuleType("antenv.axon_hooks")
    state = {"hook": hook}
    mod.get_axon_ntff_profile_hook = lambda: state["hook"]
    mod.set_axon_ntff_profile_hook = lambda h: state.update(hook=h)
    sys.modules["antenv.axon_hooks"] = mod
    import antenv

    antenv.axon_hooks = mod


def _build(causal: bool, has_padding: bool):
    nc = bacc.Bacc("TRN2", target_bir_lowering=False, debug=False, num_devices=8)
    use_dr = OPTS["use_dr"]

    # q/k inputs prepacked host-side as slice-PAIRS so each partition line
    # is 4KB (descriptor-rate bound otherwise)
    if use_dr:
        qk_shape = [2 * P, 2, EC2, 2, QW]
        qk_tile = [P, 2, EC2, 2, QW]
    else:
        qk_shape = [2 * P, 2, EC, QW]
        qk_tile = [P, 2, EC, QW]
    xq_d = nc.dram_tensor("xq", qk_shape, FP8, kind="ExternalInput")
    xk_d = nc.dram_tensor("xk", qk_shape, FP8, kind="ExternalInput")
    xv_d = nc.dram_tensor("xv", [NQB * P, EC, QW], BF16, kind="ExternalInput")
    if use_dr:
        wqk_d = nc.dram_tensor("wqk", [P, EC2, 2, 2 * DH], FP8, kind="ExternalInput")
    else:
        wqk_d = nc.dram_tensor("wqk", [P, EC, 2 * DH], FP8, kind="ExternalInput")
    wv_d = nc.dram_tensor("wv", [P, EC, DH], BF16, kind="ExternalInput")
    if has_padding:
        km_d = nc.dram_tensor("kmask", [KC, P], F32, kind="ExternalInput")
    # out rows = qb*128 + p, col block i -> full row q = qb*512 + i*128 + p
    # (host unpermutes); per-partition DMA lines are contiguous
    out_d = nc.dram_tensor("out", [NQB * P, NQB * DH], F32, kind="ExternalOutput")

    with tile.TileContext(nc) as tc:
        with (
            tc.tile_pool(name="const", bufs=1) as cpool,
            tc.tile_pool(name="x", bufs=4) as xpool,
            tc.tile_pool(name="big", bufs=1) as bigpool,
            tc.tile_pool(name="p", bufs=OPTS["ppool"]) as ppool,
            tc.tile_pool(name="epi", bufs=2) as epipool,
            tc.tile_pool(name="osb", bufs=2) as opool_sb,
            tc.tile_pool(name="o", bufs=2, space="PSUM") as opool,
            tc.tile_pool(name="s", bufs=2, space="PSUM") as spool,
            tc.tile_pool(name="j", bufs=2, space="PSUM") as jpool,
        ):
            # --- ACT warmup: load the exp table during the DMA window ---
            warm = cpool.tile([P, 1], F32)
            nc.vector.memset(warm[:], 0.0)
            nc.scalar.activation(warm[:], warm[:], mybir.ActivationFunctionType.Exp)

            # consts emitted before any DMA issue so their engines (vector
            # memset, gpsimd affine_select) aren't stuck behind dma issues
            wjunk = cpool.tile([P, P], BF16)
            nc.vector.memset(wjunk[:], 0.25)
            ident = cpool.tile([P, P], F32)
            masks.make_identity(nc, ident[:])
            tri = cpool.tile([P, P], BF16)
            masks.make_upper_triangular(nc, tri[:], val=1.0, diag=True)

            if use_dr:
                wqk_sb = cpool.tile([P, EC2, 2, 2 * DH], FP8)
            else:
                wqk_sb = cpool.tile([P, EC, 2 * DH], FP8)
            wv_sb = cpool.tile([P, EC, DH], BF16)
            if has_padding:
                km_sb = cpool.tile([P, KC], F32)
                nc.sync.dma_start(km_sb[:], km_d.ap().transpose([1, 0]))

            xq_sb, xk_sb, xv_sb = {}, {}, {}

            def alloc_qk(pair):
                xq_sb[pair] = xpool.tile(qk_tile, FP8, tag="x", name=f"xq{pair}")
                xk_sb[pair] = xpool.tile(qk_tile, FP8, tag="x", name=f"xk{pair}")

            def alloc_v(s):
                xv_sb[s] = xpool.tile([P, EC, QW], BF16, tag="x", name=f"xv{s}")

            def corner(t):
                nd = len(t.shape)
                return t[tuple([slice(0, 1)] * (nd - 1) + [slice(0, 16)])]

            def corner_gate(eng, dst_tile, src_tile):
                """Tiny DMA writing dst's corner after src's transfer has
                fully landed -> dst's main DMA gets a WAW dep that fires on
                TRANSFER completion (not proj reads)."""
                eng.dma_start(corner(dst_tile), corner(src_tile))

            # DMA sequencing: every transfer carries a real data dep.
            #   wave0 (free): weights at queue heads + q01 + k01
            #   v0: corner-gated on q01's transfer; v1 on v0's
            #   q23/k23: x-pool buf reuse (WAR dep on proj reads of q01/k01)
            #   v2/v3: buf reuse on v0/v1 (WAR on proj_v reads)
            alloc_qk(0)
            nc.sync.dma_start(wqk_sb[:], wqk_d.ap())
            nc.scalar.dma_start(wv_sb[:], wv_d.ap())
            nc.scalar.dma_start(xq_sb[0][:], xq_d.ap()[0:P])
            nc.sync.dma_start(xk_sb[0][:], xk_d.ap()[0:P])
            alloc_v(0)
            alloc_v(1)
            # fp8 -> bf16 corner write must go through gpsimd (the only
            # engine whose DMAs may cast); the WAW dep works cross-queue
            corner_gate(nc.gpsimd, xv_sb[0], xq_sb[0])
            nc.sync.dma_start(xv_sb[0][:], xv_d.ap()[0:P])
            corner_gate(nc.sync, xv_sb[1], xv_sb[0])
            nc.gpsimd.dma_start(xv_sb[1][:], xv_d.ap()[P:2 * P])

            # --- PE warmup: HAM clock-gates the PE array to 1.2 GHz until
            # ~3.4us of sustained matmul activity ---
            if OPTS["pe_warm"]:
                wps = jpool.tile([P, QW], F32, tag="j", name="warmps")
                for _ in range(OPTS["pe_warm"]):
                    nc.tensor.matmul(
                        wps[:, :P], wjunk[:], wjunk[:],
                        start=True, stop=True, skip_group_check=True,
                    )

            qt = bigpool.tile([P, N], BF16, tag="qt")   # rows 0-63 QT, 64-127 dup
            kt = bigpool.tile([P, N], BF16, tag="kt")
            vt = bigpool.tile([DH, N], F32, tag="vt")
            v_sb = bigpool.tile([P, KC, DH + 1], BF16, tag="vn")

            def proj_qk(s):
                sl = slice(s * QW, (s + 1) * QW)
                for tname, xd, big in (("q", xq_sb, qt), ("k", xk_sb, kt)):
                    x_t = xd[s // 2][:, s % 2]
                    ps = jpool.tile([P, QW], F32, tag="j", name=f"{tname}p{s}")
                    if use_dr:
                        for c in range(EC2):
                            nc.tensor.matmul(
                                ps[:],
                                wqk_sb[:, c],
                                x_t[:, c],
                                start=(c == 0),
                                stop=(c == EC2 - 1),
                                perf_mode=DR,
                            )
                    else:
                        for c in range(EC):
                            nc.tensor.matmul(
                                ps[:],
                                wqk_sb[:, c, :],
                                x_t[:, c, :],
                                start=(c == 0),
                                stop=(c == EC - 1),
                            )
                    if s == 0 and tname == "k":
                        # slice-0 K copy on the (still idle) ACT engine so
                        # the q and k copies run in parallel at startup
                        nc.scalar.activation(
                            big[:, sl], ps[:],
                            mybir.ActivationFunctionType.Copy,
                        )
                    else:
                        nc.vector.tensor_copy(big[:, sl], ps[:])

            def proj_v(s):
                sl = slice(s * QW, (s + 1) * QW)
                ps = jpool.tile([P, QW], F32, tag="j", name=f"vp{s}")
                for c in range(EC):
                    nc.tensor.matmul(
                        ps[:DH, :],
                        wv_sb[:, c, :],
                        xv_sb[s][:, c, :],
                        start=(c == 0),
                        stop=(c == EC - 1),
                    )
                nc.vector.tensor_copy(vt[:, sl], ps[:DH, :])
                # V natural tiles: PE transpose + ones column (row-sums of P
                # come free as row 64 of the PV matmul)
                vtp = jpool.tile([P, NQB, DH + 1], F32, tag="j", name=f"vt{s}")
                for i in range(NQB):
                    j = s * NQB + i
                    nc.tensor.transpose(
                        vtp[:, i, :DH], vt[:, j * P:(j + 1) * P], ident[:DH, :DH]
                    )
                nc.vector.memset(vtp[:, :, DH], 1.0)
                nc.vector.tensor_copy(v_sb[:, s * NQB:(s + 1) * NQB, :], vtp[:])

            # --- attention, q-block outer; k-chunk pairs row-packed ---
            def emit_s_pair(qb, t, p_tiles):
                j0, j1 = 2 * t, 2 * t + 1
                s_ps = spool.tile([P, 2 * QW], F32, tag="s", name=f"s{qb}_{t}")
                # exp reads contiguously from q_off0; j1 computes full width
                # so no unwritten PSUM is read
                q_off0 = max(0, j0 * P - qb * QW) if causal else 0
                nc.tensor.matmul(
                    s_ps[:, q_off0:QW],
                    kt[0:DH, j0 * P:(j0 + 1) * P],
                    qt[0:DH, qb * QW + q_off0:(qb + 1) * QW],
                    start=True, stop=True,
                )
                nc.tensor.matmul(
                    s_ps[:, QW:],
                    kt[DH:P, j1 * P:(j1 + 1) * P],
                    qt[DH:P, qb * QW:(qb + 1) * QW],
                    start=True, stop=True,
                )
                p_sb = ppool.tile([P, 2 * QW], BF16, tag="p", name=f"p{qb}_{t}")
                nc.scalar.activation(
                    p_sb[:, q_off0:],
                    s_ps[:, q_off0:],
                    mybir.ActivationFunctionType.Exp,
                    scale=EXP_SCALE,
                )
                if causal:
                    # late q-blocks' diag masks go to the (by then idle)
                    # gpsimd engine to offload DVE
                    teng = nc.gpsimd if qb >= 2 else nc.vector
                    for idx, j in enumerate((j0, j1)):
                        if j // NQB == qb:
                            # diagonal 128x128 block: keep q_loc >= k_loc
                            lo = idx * QW + (j % NQB) * P
                            teng.tensor_mul(
                                p_sb[:, lo:lo + P], p_sb[:, lo:lo + P], tri[:]
                            )
                if has_padding:
                    for idx, j in enumerate((j0, j1)):
                        off = max(0, j * P - qb * QW) if causal else 0
                        nc.vector.tensor_scalar_mul(
                            p_sb[:, idx * QW + off:(idx + 1) * QW],
                            p_sb[:, idx * QW + off:(idx + 1) * QW],
                            km_sb[:, j:j + 1],
                        )
                p_tiles[t] = p_sb

            def emit_pv(qb, t, o_ps, p_tiles, first, last):
                p_sb = p_tiles.pop(t)
                for idx, j in enumerate((2 * t, 2 * t + 1)):
                    q_off = max(0, j * P - qb * QW) if causal else 0
                    nc.tensor.matmul(
                        o_ps[:, q_off:QW],
                        v_sb[:, j, :],
                        p_sb[:, idx * QW + q_off:(idx + 1) * QW],
                        start=(first and idx == 0),
                        stop=(last and idx == 1),
                    )

            def epilogue(qb, o_ps):
                oT = epipool.tile([DH + 1, QW], F32, tag="ot")
                nc.vector.tensor_copy(oT[:], o_ps[:])
                etp = jpool.tile([P, NQB, DH + 1], F32, tag="j", name=f"et{qb}")
                for i in range(NQB):
                    nc.tensor.transpose(
                        etp[:, i, :], oT[:, i * P:(i + 1) * P],
                        ident[:DH + 1, :DH + 1],
                    )
                recip = epipool.tile([P, NQB], F32, tag="recip")
                nc.vector.reciprocal(recip[:], etp[:, :, DH])
                o_sb = opool_sb.tile([P, NQB, DH], F32, tag="osb",
                                     name=f"osb{qb}")
                for i in range(NQB):
                    nc.vector.tensor_scalar_mul(
                        o_sb[:, i, :], etp[:, i, :DH], recip[:, i:i + 1]
                    )
                nc.sync.dma_start(
                    out_d.ap()[qb * P:(qb + 1) * P, :], o_sb[:]
                )

            def dma_v(s, eng):
                alloc_v(s)
                eng.dma_start(xv_sb[s][:], xv_d.ap()[s * P:(s + 1) * P])

            def dma_qk(pair):
                alloc_qk(pair)
                nc.sync.dma_start(xq_sb[pair][:], xq_d.ap()[pair * P:(pair + 1) * P])
                nc.gpsimd.dma_start(xk_sb[pair][:], xk_d.ap()[pair * P:(pair + 1) * P])

            def attn_qb(qb, npairs, t_projv, reverse, post_projv=None):
                order = list(range(npairs))
                if reverse:
                    order.reverse()
                o_ps = opool.tile([DH + 1, QW], F32, tag="o", name=f"o{qb}")
                p_tiles = {}
                for pos, t in enumerate(order):
                    emit_s_pair(qb, t, p_tiles)
                    if t == t_projv:
                        proj_v(qb)
                        if post_projv is not None:
                            post_projv()
                    if pos > 0:
                        emit_pv(qb, order[pos - 1], o_ps, p_tiles,
                                first=(pos == 1), last=False)
                emit_pv(qb, order[-1], o_ps, p_tiles,
                        first=(npairs == 1), last=True)
                epilogue(qb, o_ps)

            # --- main emission: proj interleaved with q-block phases;
            # software-pipelined S/PV so PE work overlaps the wide exps ---
            if causal:
                proj_qk(0)
                proj_qk(1)
                dma_qk(1)   # slice pair 2/3: reuses q01/k01 bufs -> gated
                            # on the proj reads just emitted
                for qb in range(NQB):
                    npairs = 2 * qb + 2
                    t_projv = npairs - 1 if qb == 0 else 2 * qb
                    # v2/v3 ride on v0/v1's bufs (gated on proj_v reads);
                    # last q-block runs pairs in reverse so its tail after
                    # the final exp is a mask-free PV
                    post = None
                    if qb == 0:
                        post = lambda: dma_v(2, nc.sync)
                    elif qb == 1:
                        post = lambda: dma_v(3, nc.gpsimd)
                    attn_qb(qb, npairs, t_projv,
                            reverse=(qb == NQB - 1), post_projv=post)
                    if qb + 2 < NQB:
                        proj_qk(qb + 2)
            else:
                proj_qk(0)
                proj_qk(1)
                dma_qk(1)
                proj_v(0)
                proj_v(1)
                dma_v(2, nc.sync)
                dma_v(3, nc.gpsimd)
                proj_qk(2)
                proj_qk(3)
                proj_v(2)
                proj_v(3)
                for qb in range(NQB):
                    attn_qb(qb, KC // 2, -1, reverse=False)

    nc.compile()
    return nc


def _get(causal: bool, has_padding: bool):
    key = (causal, has_padding)
    if key not in _BUILD_CACHE:
        _BUILD_CACHE[key] = _build(causal, has_padding)
    return _BUILD_CACHE[key]


def _pack_x(x_t: np.ndarray, dtype) -> np.ndarray:
    """[D, N] -> SBUF tile layout [(slice p), chunk, qw]."""
    return np.ascontiguousarray(
        x_t.reshape(EC, P, NQB, QW).transpose(2, 1, 0, 3)
        .reshape(NQB * P, EC, QW).astype(dtype)
    )


def _pack_qk(x_t: np.ndarray, dtype) -> np.ndarray:
    """[D, N] -> slice-pair layout (4KB partition lines)."""
    if OPTS["use_dr"]:
        # d = c*256 + ko*128 + ki ; [pair, ki, sip, c, ko, qw]
        return np.ascontiguousarray(
            x_t.reshape(EC2, 2, P, 2, 2, QW).transpose(3, 2, 4, 0, 1, 5)
            .reshape(2 * P, 2, EC2, 2, QW).astype(dtype)
        )
    return np.ascontiguousarray(
        x_t.reshape(EC, P, 2, 2, QW).transpose(2, 1, 3, 0, 4)
        .reshape(2 * P, 2, EC, QW).astype(dtype)
    )


def run(key_input, query_input, value_input, padding_mask, masked_attention,
        W_key, W_query=None, W_value=None, trace=False, **_ignored):
    key_input = np.asarray(key_input, dtype=np.float32)
    query_input = np.asarray(query_input, dtype=np.float32)
    value_input = np.asarray(value_input, dtype=np.float32)
    padding_mask = np.asarray(padding_mask)
    W_key = np.asarray(W_key, dtype=np.float32)

    B = key_input.shape[0]
    causal = bool(int(np.asarray(masked_attention)))
    has_padding = bool(padding_mask.any())
    nc = _get(causal, has_padding)

    bf = ml_dtypes.bfloat16
    f8 = ml_dtypes.float8_e4m3fn
    wcat = np.concatenate([W_key, W_key], axis=1) * WS
    if OPTS["use_dr"]:
        wqk = np.ascontiguousarray(
            wcat.reshape(EC2, 2, P, 2 * DH).transpose(2, 0, 1, 3).astype(f8)
        )
    else:
        wqk = np.ascontiguousarray(
            wcat.reshape(EC, P, 2 * DH).transpose(1, 0, 2).astype(f8)
        )
    wv = np.ascontiguousarray(
        W_key.reshape(EC, P, DH).transpose(1, 0, 2).astype(bf)
    )
    in_maps = []
    for b in range(B):
        m = {
            "xq": _pack_qk(query_input[b].T, f8),
            "xk": _pack_qk(key_input[b].T, f8),
            "xv": _pack_x(value_input[b].T, bf),
            "wqk": wqk,
            "wv": wv,
        }
        if has_padding:
            km = (~padding_mask[b].reshape(N)).astype(np.float32)
            m["kmask"] = np.ascontiguousarray(km.reshape(KC, P))
        in_maps.append(m)

    if trace:
        _ensure_ntff_hook()
    res = run_bass_kernel_spmd(nc, in_maps, core_ids=list(range(B)), trace=trace)
    outs = []
    for b in range(B):
        o = np.asarray(res.results[b]["out"])  # [(qb p), (i d)]
        o = o.reshape(NQB, P, NQB, DH).transpose(0, 2, 1, 3).reshape(N, DH)
        outs.append(o)
    out = np.stack(outs, axis=0)
    return out.astype(np.float32), res


def kernel(**inputs) -> np.ndarray:
    out, _ = run(**inputs)
    return out


# revision 34
# speedup vs baseline: 1.0838x; 1.0838x over previous
"""Trainium2 Bass kernel for nn_AttentionHead (B=8, N=2048, D=512, d=64).

Reference semantics (faithful to the torch original):
    K = key_input   @ W_key        # note: W_key used for Q, K AND V
    Q = query_input @ W_key
    V = value_input @ W_key
    S = Q @ K^T / sqrt(512)        # scaled by INPUT dim, not head dim
    S = mask(padding), causal-mask if masked_attention
    out = softmax(S) @ V

Sharding: pure data parallelism over batch — core b computes batch element b.
No collectives. Host-side prep is layout only (transpose + dtype cast +
output unpermute); every FLOP of the math runs on-device.

Device algorithm (per core), v9:
  - xq/xk stream in fp8e4 (W_qk prescaled x16 host-side, descale folded into
    the exp scale); projections use DoubleRow perf mode (256-deep
    contraction, half the matmuls); xv stays bf16 to protect output precision
  - DMA completion is DESCRIPTOR-bound (~130ns per partition line), so all
    inputs are host-packed into 4KB-per-partition lines: q/k slices ride in
    PAIRS, v slices whole. Priority is enforced with real data deps only
    (corner-gate DMAs + x-pool buf reuse) — the rings round-robin across
    all in-flight transfers and the tile scheduler reorders issue order
  - q-block-outer attention: per 512-wide q-block, k-chunks processed in
    row-packed pairs (two 64-deep S matmuls concurrently in disjoint PE row
    groups) writing one [128, 1024] PSUM tile; ONE wide exp per pair on ACT
    (amortizes the ~293ns ACTIVATE overhead); p_sb pool sized one-per-group
    so no recycle deps serialize the ACT queue
  - diagonal k-chunks compute S full-width (free under row-pairing) so the
    wide exp read is contiguous; only 128-wide diagonal blocks get the
    upper-triangular mask multiply (DVE early, gpsimd for late q-blocks);
    PV uses exact causal widths
  - O.T [65, q] accumulated in PSUM over k-chunks (ones column appended to
    V-natural gives softmax denominators as row 64); per-q-block epilogue:
    PE-transpose, reciprocal, scale, one batched DMA out
  - PSUM: 2x [128,1024] S (4 banks) + 2x [65,512] O (2) + 2x [128,512]
    proj/transpose (2) = 8 banks
"""

import math

import numpy as np
import ml_dtypes

import concourse.bass as bass
import concourse.tile as tile
from concourse import bacc, mybir
from concourse import masks
from concourse.bass_utils import run_bass_kernel_spmd

P = 128            # partitions / k-chunk size
N = 2048           # sequence length
D = 512            # embedding dim
DH = 64            # head dim
EC = D // P        # 4 e-chunks for the (bf16) V projection contraction
EC2 = D // (2 * P)  # 2 double-row chunks for the fp8 Q/K projections
KC = N // P        # 16 k-chunks
QW = 512           # q block width
NQB = N // QW      # 4 q blocks / n slices
WS = 16.0          # host-side W_qk prescale (fp8 range use)
SCALE = 1.0 / math.sqrt(float(D))
EXP_SCALE = SCALE / (WS * WS)

BF16 = mybir.dt.bfloat16
FP8 = mybir.dt.float8e4
F32 = mybir.dt.float32
DR = mybir.MatmulPerfMode.DoubleRow

_BUILD_CACHE = {}

OPTS = {
    "pe_warm": 30,     # dummy matmuls at t=0 to lift the HAM clock gate
    "ppool": 20,       # p_sb wide-tile buffers: one per group, so no recycle
                       # deps ever land as EVENT_SEMAPHORE waits on ACT
    "use_dr": True,    # DoubleRow fp8 projections
}


def _ensure_ntff_hook():
    """Install the antenv.axon_hooks shim so trace=True works under axon."""
    try:
        import antenv.axon_hooks  # noqa: F401
        return
    except ImportError:
        pass
    import sys
    import types

    try:
        from trn_agent_boot.trn_boot import _ntff_profile_via_ctypes
        hook = _ntff_profile_via_ctypes("/opt/axon/libaxon_pjrt.so")
    except Exception:
        hook = None
    mod = types.ModuleType("antenv.axon_hooks")
    state = {"hook": hook}
    mod.get_axon_ntff_profile_hook = lambda: state["hook"]
    mod.set_axon_ntff_profile_hook = lambda h: state.update(hook=h)
    sys.modules["antenv.axon_hooks"] = mod
    import antenv

    antenv.axon_hooks = mod


def _build(causal: bool, has_padding: bool):
    nc = bacc.Bacc("TRN2", target_bir_lowering=False, debug=False, num_devices=8)
    use_dr = OPTS["use_dr"]

    # q/k inputs prepacked host-side as slice-PAIRS so each partition line
    # is 4KB (descriptor-rate bound otherwise)
    if use_dr:
        qk_shape = [2 * P, 2, EC2, 2, QW]
        qk_tile = [P, 2, EC2, 2, QW]
    else:
        qk_shape = [2 * P, 2, EC, QW]
        qk_tile = [P, 2, EC, QW]
    xq_d = nc.dram_tensor("xq", qk_shape, FP8, kind="ExternalInput")
    xk_d = nc.dram_tensor("xk", qk_shape, FP8, kind="ExternalInput")
    xv_d = nc.dram_tensor("xv", [NQB * P, EC, QW], BF16, kind="ExternalInput")
    if use_dr:
        wqk_d = nc.dram_tensor("wqk", [P, EC2, 2, 2 * DH], FP8, kind="ExternalInput")
    else:
        wqk_d = nc.dram_tensor("wqk", [P, EC, 2 * DH], FP8, kind="ExternalInput")
    wv_d = nc.dram_tensor("wv", [P, EC, DH], BF16, kind="ExternalInput")
    if has_padding:
        km_d = nc.dram_tensor("kmask", [KC, P], F32, kind="ExternalInput")
    # out rows = qb*128 + p, col block i -> full row q = qb*512 + i*128 + p
    # (host unpermutes); per-partition DMA lines are contiguous
    out_d = nc.dram_tensor("out", [NQB * P, NQB * DH], F32, kind="ExternalOutput")

    with tile.TileContext(nc) as tc:
        with (
            tc.tile_pool(name="const", bufs=1) as cpool,
            tc.tile_pool(name="x", bufs=4) as xpool,
            tc.tile_pool(name="big", bufs=1) as bigpool,
            tc.tile_pool(name="p", bufs=OPTS["ppool"]) as ppool,
            tc.tile_pool(name="epi", bufs=2) as epipool,
            tc.tile_pool(name="osb", bufs=2) as opool_sb,
            tc.tile_pool(name="o", bufs=2, space="PSUM") as opool,
            tc.tile_pool(name="s", bufs=2, space="PSUM") as spool,
            tc.tile_pool(name="j", bufs=2, space="PSUM") as jpool,
        ):
            # --- ACT warmup: load the exp table during the DMA window ---
            warm = cpool.tile([P, 1], F32)
            nc.vector.memset(warm[:], 0.0)
            nc.scalar.activation(warm[:], warm[:], mybir.ActivationFunctionType.Exp)

            # consts emitted before any DMA issue so their engines (vector
            # memset, gpsimd affine_select) aren't stuck behind dma issues
            wjunk = cpool.tile([P, P], BF16)
            nc.vector.memset(wjunk[:], 0.25)
            ident = cpool.tile([P, P], F32)
            masks.make_identity(nc, ident[:])
            tri = cpool.tile([P, P], BF16)
            masks.make_upper_triangular(nc, tri[:], val=1.0, diag=True)

            if use_dr:
                wqk_sb = cpool.tile([P, EC2, 2, 2 * DH], FP8)
            else:
                wqk_sb = cpool.tile([P, EC, 2 * DH], FP8)
            wv_sb = cpool.tile([P, EC, DH], BF16)
            if has_padding:
                km_sb = cpool.tile([P, KC], F32)
                nc.sync.dma_start(km_sb[:], km_d.ap().transpose([1, 0]))

            xq_sb, xk_sb, xv_sb = {}, {}, {}

            def alloc_qk(pair):
                xq_sb[pair] = xpool.tile(qk_tile, FP8, tag="x", name=f"xq{pair}")
                xk_sb[pair] = xpool.tile(qk_tile, FP8, tag="x", name=f"xk{pair}")

            def alloc_v(s):
                xv_sb[s] = xpool.tile([P, EC, QW], BF16, tag="x", name=f"xv{s}")

            def corner(t):
                nd = len(t.shape)
                return t[tuple([slice(0, 1)] * (nd - 1) + [slice(0, 16)])]

            def corner_gate(eng, dst_tile, src_tile):
                """Tiny DMA writing dst's corner after src's transfer has
                fully landed -> dst's main DMA gets a WAW dep that fires on
                TRANSFER completion (not proj reads)."""
                eng.dma_start(corner(dst_tile), corner(src_tile))

            # DMA sequencing: every transfer carries a real data dep.
            #   wave0 (free): weights at queue heads + q01 + k01
            #   v0: corner-gated on q01's transfer; v1 on v0's
            #   q23/k23: x-pool buf reuse (WAR dep on proj reads of q01/k01)
            #   v2/v3: buf reuse on v0/v1 (WAR on proj_v reads)
            alloc_qk(0)
            nc.sync.dma_start(wqk_sb[:], wqk_d.ap())
            nc.scalar.dma_start(wv_sb[:], wv_d.ap())
            nc.scalar.dma_start(xq_sb[0][:], xq_d.ap()[0:P])
            nc.sync.dma_start(xk_sb[0][:], xk_d.ap()[0:P])
            alloc_v(0)
            alloc_v(1)
            # v0/v1 gated on the tiny WEIGHT transfers (done ~3us before
            # q01/k01): late enough to give q/k a head start, early enough
            # that the PV path isn't starved. fp8->bf16 corner writes must
            # go through gpsimd (the only engine whose DMAs may cast).
            corner_gate(nc.sync, xv_sb[0], wv_sb)
            nc.sync.dma_start(xv_sb[0][:], xv_d.ap()[0:P])
            corner_gate(nc.gpsimd, xv_sb[1], wqk_sb)
            nc.gpsimd.dma_start(xv_sb[1][:], xv_d.ap()[P:2 * P])

            # --- PE warmup: HAM clock-gates the PE array to 1.2 GHz until
            # ~3.4us of sustained matmul activity ---
            if OPTS["pe_warm"]:
                wps = jpool.tile([P, QW], F32, tag="j", name="warmps")
                for _ in range(OPTS["pe_warm"]):
                    nc.tensor.matmul(
                        wps[:, :P], wjunk[:], wjunk[:],
                        start=True, stop=True, skip_group_check=True,
                    )

            qt = bigpool.tile([P, N], BF16, tag="qt")   # rows 0-63 QT, 64-127 dup
            kt = bigpool.tile([P, N], BF16, tag="kt")
            vt = bigpool.tile([DH, N], F32, tag="vt")
            v_sb = bigpool.tile([P, KC, DH + 1], BF16, tag="vn")

            def proj_qk(s):
                sl = slice(s * QW, (s + 1) * QW)
                for tname, xd, big in (("q", xq_sb, qt), ("k", xk_sb, kt)):
                    x_t = xd[s // 2][:, s % 2]
                    ps = jpool.tile([P, QW], F32, tag="j", name=f"{tname}p{s}")
                    if use_dr:
                        for c in range(EC2):
                            nc.tensor.matmul(
                                ps[:],
                                wqk_sb[:, c],
                                x_t[:, c],
                                start=(c == 0),
                                stop=(c == EC2 - 1),
                                perf_mode=DR,
                            )
                    else:
                        for c in range(EC):
                            nc.tensor.matmul(
                                ps[:],
                                wqk_sb[:, c, :],
                                x_t[:, c, :],
                                start=(c == 0),
                                stop=(c == EC - 1),
                            )
                    if s == 0 and tname == "k":
                        # slice-0 K copy on the (still idle) ACT engine so
                        # the q and k copies run in parallel at startup
                        nc.scalar.activation(
                            big[:, sl], ps[:],
                            mybir.ActivationFunctionType.Copy,
                        )
                    else:
                        nc.vector.tensor_copy(big[:, sl], ps[:])

            def proj_v(s):
                sl = slice(s * QW, (s + 1) * QW)
                ps = jpool.tile([P, QW], F32, tag="j", name=f"vp{s}")
                for c in range(EC):
                    nc.tensor.matmul(
                        ps[:DH, :],
                        wv_sb[:, c, :],
                        xv_sb[s][:, c, :],
                        start=(c == 0),
                        stop=(c == EC - 1),
                    )
                nc.vector.tensor_copy(vt[:, sl], ps[:DH, :])
                # V natural tiles: PE transpose + ones column (row-sums of P
                # come free as row 64 of the PV matmul)
                vtp = jpool.tile([P, NQB, DH + 1], F32, tag="j", name=f"vt{s}")
                for i in range(NQB):
                    j = s * NQB + i
                    nc.tensor.transpose(
                        vtp[:, i, :DH], vt[:, j * P:(j + 1) * P], ident[:DH, :DH]
                    )
                nc.vector.memset(vtp[:, :, DH], 1.0)
                nc.vector.tensor_copy(v_sb[:, s * NQB:(s + 1) * NQB, :], vtp[:])

            # --- attention, q-block outer; k-chunk pairs row-packed ---
            def emit_s_pair(qb, t, p_tiles):
                j0, j1 = 2 * t, 2 * t + 1
                s_ps = spool.tile([P, 2 * QW], F32, tag="s", name=f"s{qb}_{t}")
                # exp reads contiguously from q_off0; j1 computes full width
                # so no unwritten PSUM is read
                q_off0 = max(0, j0 * P - qb * QW) if causal else 0
                nc.tensor.matmul(
                    s_ps[:, q_off0:QW],
                    kt[0:DH, j0 * P:(j0 + 1) * P],
                    qt[0:DH, qb * QW + q_off0:(qb + 1) * QW],
                    start=True, stop=True,
                )
                nc.tensor.matmul(
                    s_ps[:, QW:],
                    kt[DH:P, j1 * P:(j1 + 1) * P],
                    qt[DH:P, qb * QW:(qb + 1) * QW],
                    start=True, stop=True,
                )
                p_sb = ppool.tile([P, 2 * QW], BF16, tag="p", name=f"p{qb}_{t}")
                nc.scalar.activation(
                    p_sb[:, q_off0:],
                    s_ps[:, q_off0:],
                    mybir.ActivationFunctionType.Exp,
                    scale=EXP_SCALE,
                )
                if causal:
                    # late q-blocks' diag masks go to the (by then idle)
                    # gpsimd engine to offload DVE
                    teng = nc.gpsimd if qb >= 2 else nc.vector
                    for idx, j in enumerate((j0, j1)):
                        if j // NQB == qb:
                            # diagonal 128x128 block: keep q_loc >= k_loc
                            lo = idx * QW + (j % NQB) * P
                            teng.tensor_mul(
                                p_sb[:, lo:lo + P], p_sb[:, lo:lo + P], tri[:]
                            )
                if has_padding:
                    for idx, j in enumerate((j0, j1)):
                        off = max(0, j * P - qb * QW) if causal else 0
                        nc.vector.tensor_scalar_mul(
                            p_sb[:, idx * QW + off:(idx + 1) * QW],
                            p_sb[:, idx * QW + off:(idx + 1) * QW],
                            km_sb[:, j:j + 1],
                        )
                p_tiles[t] = p_sb

            def emit_pv(qb, t, o_ps, p_tiles, first, last):
                p_sb = p_tiles.pop(t)
                for idx, j in enumerate((2 * t, 2 * t + 1)):
                    q_off = max(0, j * P - qb * QW) if causal else 0
                    nc.tensor.matmul(
                        o_ps[:, q_off:QW],
                        v_sb[:, j, :],
                        p_sb[:, idx * QW + q_off:(idx + 1) * QW],
                        start=(first and idx == 0),
                        stop=(last and idx == 1),
                    )

            def epilogue(qb, o_ps):
                oT = epipool.tile([DH + 1, QW], F32, tag="ot")
                nc.vector.tensor_copy(oT[:], o_ps[:])
                etp = jpool.tile([P, NQB, DH + 1], F32, tag="j", name=f"et{qb}")
                for i in range(NQB):
                    nc.tensor.transpose(
                        etp[:, i, :], oT[:, i * P:(i + 1) * P],
                        ident[:DH + 1, :DH + 1],
                    )
                recip = epipool.tile([P, NQB], F32, tag="recip")
                nc.vector.reciprocal(recip[:], etp[:, :, DH])
                o_sb = opool_sb.tile([P, NQB, DH], F32, tag="osb",
                                     name=f"osb{qb}")
                for i in range(NQB):
                    nc.vector.tensor_scalar_mul(
                        o_sb[:, i, :], etp[:, i, :DH], recip[:, i:i + 1]
                    )
                nc.sync.dma_start(
                    out_d.ap()[qb * P:(qb + 1) * P, :], o_sb[:]
                )

            def dma_v(s, eng):
                alloc_v(s)
                eng.dma_start(xv_sb[s][:], xv_d.ap()[s * P:(s + 1) * P])

            def dma_qk(pair):
                alloc_qk(pair)
                nc.sync.dma_start(xq_sb[pair][:], xq_d.ap()[pair * P:(pair + 1) * P])
                nc.gpsimd.dma_start(xk_sb[pair][:], xk_d.ap()[pair * P:(pair + 1) * P])

            def attn_qb(qb, npairs, t_projv, reverse, post_projv=None):
                order = list(range(npairs))
                if reverse:
                    order.reverse()
                o_ps = opool.tile([DH + 1, QW], F32, tag="o", name=f"o{qb}")
                p_tiles = {}
                for pos, t in enumerate(order):
                    emit_s_pair(qb, t, p_tiles)
                    if t == t_projv:
                        proj_v(qb)
                        if post_projv is not None:
                            post_projv()
                    if pos > 0:
                        emit_pv(qb, order[pos - 1], o_ps, p_tiles,
                                first=(pos == 1), last=False)
                emit_pv(qb, order[-1], o_ps, p_tiles,
                        first=(npairs == 1), last=True)
                epilogue(qb, o_ps)

            # --- main emission: proj interleaved with q-block phases;
            # software-pipelined S/PV so PE work overlaps the wide exps ---
            if causal:
                proj_qk(0)
                proj_qk(1)
                dma_qk(1)   # slice pair 2/3: reuses q01/k01 bufs -> gated
                            # on the proj reads just emitted
                for qb in range(NQB):
                    npairs = 2 * qb + 2
                    t_projv = npairs - 1 if qb == 0 else 2 * qb
                    # v2/v3 ride on v0/v1's bufs (gated on proj_v reads);
                    # last q-block runs pairs in reverse so its tail after
                    # the final exp is a mask-free PV
                    post = None
                    if qb == 0:
                        post = lambda: dma_v(2, nc.sync)
                    elif qb == 1:
                        post = lambda: dma_v(3, nc.gpsimd)
                    attn_qb(qb, npairs, t_projv,
                            reverse=(qb == NQB - 1), post_projv=post)
                    if qb + 2 < NQB:
                        proj_qk(qb + 2)
            else:
                proj_qk(0)
                proj_qk(1)
                dma_qk(1)
                proj_v(0)
                proj_v(1)
                dma_v(2, nc.sync)
                dma_v(3, nc.gpsimd)
                proj_qk(2)
                proj_qk(3)
                proj_v(2)
                proj_v(3)
                for qb in range(NQB):
                    attn_qb(qb, KC // 2, -1, reverse=False)

    nc.compile()
    return nc


def _get(causal: bool, has_padding: bool):
    key = (causal, has_padding)
    if key not in _BUILD_CACHE:
        _BUILD_CACHE[key] = _build(causal, has_padding)
    return _BUILD_CACHE[key]


def _pack_x(x_t: np.ndarray, dtype) -> np.ndarray:
    """[D, N] -> SBUF tile layout [(slice p), chunk, qw]."""
    return np.ascontiguousarray(
        x_t.reshape(EC, P, NQB, QW).transpose(2, 1, 0, 3)
        .reshape(NQB * P, EC, QW).astype(dtype)
    )


def _pack_qk(x_t: np.ndarray, dtype) -> np.ndarray:
    """[D, N] -> slice-pair layout (4KB partition lines)."""
    if OPTS["use_dr"]:
        # d = c*256 + ko*128 + ki ; [pair, ki, sip, c, ko, qw]
        return np.ascontiguousarray(
            x_t.reshape(EC2, 2, P, 2, 2, QW).transpose(3, 2, 4, 0, 1, 5)
            .reshape(2 * P, 2, EC2, 2, QW).astype(dtype)
        )
    return np.ascontiguousarray(
        x_t.reshape(EC, P, 2, 2, QW).transpose(2, 1, 3, 0, 4)
        .reshape(2 * P, 2, EC, QW).astype(dtype)
    )


def run(key_input, query_input, value_input, padding_mask, masked_attention,
        W_key, W_query=None, W_value=None, trace=False, **_ignored):
    key_input = np.asarray(key_input, dtype=np.float32)
    query_input = np.asarray(query_input, dtype=np.float32)
    value_input = np.asarray(value_input, dtype=np.float32)
    padding_mask = np.asarray(padding_mask)
    W_key = np.asarray(W_key, dtype=np.float32)

    B = key_input.shape[0]
    causal = bool(int(np.asarray(masked_attention)))
    has_padding = bool(padding_mask.any())
    nc = _get(causal, has_padding)

    bf = ml_dtypes.bfloat16
    f8 = ml_dtypes.float8_e4m3fn
    wcat = np.concatenate([W_key, W_key], axis=1) * WS
    if OPTS["use_dr"]:
        wqk = np.ascontiguousarray(
            wcat.reshape(EC2, 2, P, 2 * DH).transpose(2, 0, 1, 3).astype(f8)
        )
    else:
        wqk = np.ascontiguousarray(
            wcat.reshape(EC, P, 2 * DH).transpose(1, 0, 2).astype(f8)
        )
    wv = np.ascontiguousarray(
        W_key.reshape(EC, P, DH).transpose(1, 0, 2).astype(bf)
    )
    in_maps = []
    for b in range(B):
        m = {
            "xq": _pack_qk(query_input[b].T, f8),
            "xk": _pack_qk(key_input[b].T, f8),
            "xv": _pack_x(value_input[b].T, bf),
            "wqk": wqk,
            "wv": wv,
        }
        if has_padding:
            km = (~padding_mask[b].reshape(N)).astype(np.float32)
            m["kmask"] = np.ascontiguousarray(km.reshape(KC, P))
        in_maps.append(m)

    if trace:
        _ensure_ntff_hook()
    res = run_bass_kernel_spmd(nc, in_maps, core_ids=list(range(B)), trace=trace)
    outs = []
    for b in range(B):
        o = np.asarray(res.results[b]["out"])  # [(qb p), (i d)]
        o = o.reshape(NQB, P, NQB, DH).transpose(0, 2, 1, 3).reshape(N, DH)
        outs.append(o)
    out = np.stack(outs, axis=0)
    return out.astype(np.float32), res


def kernel(**inputs) -> np.ndarray:
    out, _ = run(**inputs)
    return out
